# revision 21
# baseline (speedup 1.0000x reference)
"""Bass/Trainium2 kernel for the GaussianRecu (Kalman-style linear scan) model.

Reference recursion (C = I, dt = 0.01), per batch b, scanned over t:
    out_t   = dt * x_t                      (emitted before update)
    x_{t+1} = x_t + dt*(A - cov_t) x_t + cov_t dy_t
    cov_{t+1} = cov_t A + A cov_t

The cov recursion is linear with spectral radius 2*rho(A); for contracting A
it underflows to EXACT fp32 zero after a few dozen steps.  Once cov == 0
exactly, the remaining recursion is exactly x <- x + dt*(A x), i.e.
    out[b, t, :] = W_t @ x*(b),   W_t = dt * G^(t-t0),  G = I + dt*A.

Device-side this is a rank-2 broadcast.  The harness tolerance is 2e-2
(max-abs-scaled), so the device pipeline runs in bf16 end to end (measured
error ~5e-3), halving HBM writes (8 MB -> 4 MB per core).

Sharding: TIME-parallel (not batch-parallel).  Each core holds ALL 128
batch rows on the PSUM partition axis and 1/8 of the (t, i) columns:

    psum[b, j] = sum_k coef[k, b] * basis[k, j]

is a K=2 matmul with the tiny [2, 128] coefficient matrix STATIONARY in the
PE array and the basis slice streaming at 1 column/cycle -> 128 output
elements/cycle.  The per-element vector-engine work collapses to a single
PSUM -> SBUF bf16 copy, alternated between ACT and DVE; sync-queue DMAs
stream the copies out.  Per core: ~65 KB input load, 4 MB output write.
"""

import numpy as np
import ml_dtypes

B, T = 128, 65536
DT32 = np.float32(0.01)
N_CORES = 8
P = 128                    # PSUM/SBUF partitions = batch rows
NCOL = 2 * T // N_CORES    # (t, i) columns per core (16384)
MM = 512                   # matmul moving free dim / one PSUM bank (f32)
GRP = 1024                 # columns per PSUM group / output DMA (2 banks)
NGRP = NCOL // GRP         # 16 groups
CPAD = 128                 # coef columns prepended to the basis plane

BF16 = ml_dtypes.bfloat16

TRACE = False          # test harness may set True to collect a HW profile
LAST_RESULTS = None    # BassKernelResults of the most recent device run

# Which engine copies PSUM group g to SBUF: A = ACT (scalar), D = DVE.
COPY_PATTERN = "ADADADADADADADAD"

_PROGRAMS = {}


def _build_program(pattern):
    import concourse.bacc as bacc
    import concourse.tile as tile
    from concourse import mybir

    bf = mybir.dt.bfloat16
    f32 = mybir.dt.float32
    nc = bacc.Bacc(
        "TRN2", target_bir_lowering=False, debug=False, num_devices=N_CORES
    )
    # r: [2, CPAD + NCOL] bf16 — cols [0:128) coef matrix (cf[k, b] =
    # x*(b)[k]), cols [128:) the basis slice for this core's t-range.
    r = nc.declare_dram_parameter("r", [2, CPAD + NCOL], bf, isOutput=False)
    out = nc.declare_dram_parameter("out", [NGRP, P, GRP], bf, isOutput=True)

    with tile.TileContext(nc) as tc:
        with (
            tc.tile_pool(name="consts", bufs=1) as consts,
            tc.psum_pool(name="ps", bufs=4) as psp,
            tc.tile_pool(name="ot", bufs=6) as otp,
        ):
            rt = consts.tile([2, CPAD + NCOL], bf)
            # 3-way load split: group 0/1's matmuls only wait for the tiny
            # first chunk; the rest streams on both HWDGE queues in parallel.
            S1 = CPAD + 2 * GRP
            S2 = S1 + (CPAD + NCOL - S1) // 2
            nc.sync.dma_start(out=rt[:, 0:S1], in_=r[:, 0:S1])
            nc.scalar.dma_start(out=rt[:, S1:S2], in_=r[:, S1:S2])
            nc.sync.dma_start(out=rt[:, S2:], in_=r[:, S2:])
            cf = rt[:, 0:CPAD]

            for g in range(NGRP):
                lo = g * GRP
                ps = psp.tile([P, GRP], f32)
                for c in range(GRP // MM):
                    nc.tensor.matmul(
                        out=ps[:, c * MM : (c + 1) * MM],
                        lhsT=cf,
                        rhs=rt[:, CPAD + lo + c * MM : CPAD + lo + (c + 1) * MM],
                        start=True,
                        stop=True,
                    )
                o = otp.tile([P, GRP], bf)
                if g == NGRP - 1:
                    # Tail taper: copy the two PSUM halves on BOTH engines
                    # concurrently so the post-last-matmul chain shortens.
                    nc.vector.tensor_scalar_mul(o[:, 0:MM], ps[:, 0:MM], 1.0)
                    nc.scalar.copy(out=o[:, MM:GRP], in_=ps[:, MM:GRP])
                elif pattern[g] == "A":
                    nc.scalar.copy(out=o[:], in_=ps[:])
                else:
                    nc.vector.tensor_scalar_mul(o[:], ps[:], 1.0)
                nc.sync.dma_start(out=out[g], in_=o[:])
    nc.compile()
    return nc


def _early_phase(dy, x0, cov0, A32):
    """Exact fp32 replica of the reference scan until cov == 0 exactly.

    Returns (early_out (B, t0, 2), xstar (B, 2), t0)."""
    x = x0.astype(np.float32).copy()
    cov = cov0.astype(np.float32).copy()
    rows = []
    t = 0
    while t < T and not np.all(cov == 0):
        rows.append(x * DT32)
        K = A32[None, :, :] - cov
        dx = np.einsum("bij,bj->bi", K, x) * DT32 + np.einsum(
            "bij,bj->bi", cov, dy[:, t, :]
        )
        cov = np.einsum("bij,jk->bik", cov, A32) + np.einsum(
            "ij,bjk->bik", A32, cov
        )
        x = x + dx
        t += 1
    early = (
        np.stack(rows, axis=1) if rows else np.zeros((B, 0, 2), np.float32)
    )
    return early.astype(np.float32), x, t


def _powers(A, n):
    """G^k for k in [0, n), fp64 block products; G = I + dt*A."""
    dtv = float(DT32)
    G = np.eye(2, dtype=np.float64) + dtv * A.astype(np.float64)
    S = 1024
    Ps = np.empty((S, 2, 2), np.float64)
    cur = np.eye(2, dtype=np.float64)
    for s in range(S):
        Ps[s] = cur
        cur = cur @ G
    GS = cur  # G^S
    M = (n + S - 1) // S
    Cs = np.empty((M, 2, 2), np.float64)
    cur = np.eye(2, dtype=np.float64)
    for m in range(M):
        Cs[m] = cur
        cur = cur @ GS
    # G^(m*S + s) = G^(m*S) @ G^s
    return np.einsum("mij,sjk->msik", Cs, Ps).reshape(M * S, 2, 2)[:n]


def kernel(dy, x0, cov0, A):
    global LAST_RESULTS
    from concourse.bass_utils import run_bass_kernel_spmd

    dy = np.ascontiguousarray(np.asarray(dy, dtype=np.float32))
    x0 = np.asarray(x0, dtype=np.float32)
    cov0 = np.asarray(cov0, dtype=np.float32)
    A32 = np.asarray(A, dtype=np.float32)
    assert dy.shape == (B, T, 2) and x0.shape == (B, 2)

    early, xstar, t0 = _early_phase(dy, x0, cov0, A32)
    K = T - t0
    dtv = float(DT32)

    # Basis: RB[k, 2t+i] = dt * (G^(t-t0))[i, k]  for t >= t0, else 0.
    RB = np.zeros((2, 2 * T), np.float64)
    if K > 0:
        Wfull = _powers(A32, K) * dtv          # (K, 2, 2) = dt*G^(t-t0)[i,k]
        RB[0, 2 * t0 :] = Wfull[:, :, 0].reshape(-1)
        RB[1, 2 * t0 :] = Wfull[:, :, 1].reshape(-1)
    RBb = RB.astype(np.float32).astype(BF16)
    cfb = np.ascontiguousarray(
        xstar.T.astype(np.float32).astype(BF16)
    )  # (2, 128)

    if COPY_PATTERN not in _PROGRAMS:
        _PROGRAMS[COPY_PATTERN] = _build_program(COPY_PATTERN)
    nc = _PROGRAMS[COPY_PATTERN]

    in_maps = []
    for c in range(N_CORES):
        plane = np.concatenate(
            [cfb, RBb[:, c * NCOL : (c + 1) * NCOL]], axis=1
        )
        in_maps.append({"r": np.ascontiguousarray(plane)})

    res = run_bass_kernel_spmd(nc, in_maps, list(range(N_CORES)), trace=TRACE)
    LAST_RESULTS = res

    parts = []
    for c in range(N_CORES):
        arr = np.asarray(res.results[c]["out"])    # (NGRP, P, GRP) bf16
        parts.append(arr.transpose(1, 0, 2).reshape(P, NCOL))
    full = (
        np.concatenate(parts, axis=1).reshape(B, T, 2).astype(np.float32)
    )
    if t0 > 0:
        full[:, :t0, :] = early
    return np.ascontiguousarray(full)


# revision 24
# speedup vs baseline: 1.0444x; 1.0444x over previous
"""Bass/Trainium2 kernel for the GaussianRecu (Kalman-style linear scan) model.

Reference recursion (C = I, dt = 0.01), per batch b, scanned over t:
    out_t   = dt * x_t                      (emitted before update)
    x_{t+1} = x_t + dt*(A - cov_t) x_t + cov_t dy_t
    cov_{t+1} = cov_t A + A cov_t

The cov recursion is linear with spectral radius 2*rho(A); for contracting A
it underflows to EXACT fp32 zero after a few dozen steps.  Once cov == 0
exactly, the remaining recursion is exactly x <- x + dt*(A x), i.e.
    out[b, t, :] = W_t @ x*(b),   W_t = dt * G^(t-t0),  G = I + dt*A.

Device-side this is a rank-2 broadcast.  The harness tolerance is 2e-2
(max-abs-scaled), so the device pipeline runs in bf16 end to end (measured
error ~5e-3), halving HBM writes (8 MB -> 4 MB per core).

Sharding: TIME-parallel (not batch-parallel).  Each core holds ALL 128
batch rows on the PSUM partition axis and 1/8 of the (t, i) columns:

    psum[b, j] = sum_k coef[k, b] * basis[k, j]

is a K=2 matmul with the tiny [2, 128] coefficient matrix STATIONARY in the
PE array and the basis slice streaming at 1 column/cycle -> 128 output
elements/cycle.  The per-element vector-engine work collapses to a single
PSUM -> SBUF bf16 copy, alternated between ACT and DVE; sync-queue DMAs
stream the copies out.  Per core: ~65 KB input load, 4 MB output write.
"""

import numpy as np
import ml_dtypes

B, T = 128, 65536
DT32 = np.float32(0.01)
N_CORES = 8
P = 128                    # PSUM/SBUF partitions = batch rows
NCOL = 2 * T // N_CORES    # (t, i) columns per core (16384)
MM = 512                   # matmul moving free dim / one PSUM bank (f32)
GRP = 1024                 # columns per PSUM group / output DMA (2 banks)
NGRP = NCOL // GRP         # 16 groups
CPAD = 128                 # coef columns prepended to the basis plane

BF16 = ml_dtypes.bfloat16

TRACE = False          # test harness may set True to collect a HW profile
LAST_RESULTS = None    # BassKernelResults of the most recent device run

# Which engine copies PSUM group g to SBUF: A = ACT (scalar), D = DVE.
COPY_PATTERN = "ADADADADADADADAD"

_PROGRAMS = {}


def _build_program(pattern):
    import concourse.bacc as bacc
    import concourse.tile as tile
    from concourse import mybir

    bf = mybir.dt.bfloat16
    f32 = mybir.dt.float32
    nc = bacc.Bacc(
        "TRN2", target_bir_lowering=False, debug=False, num_devices=N_CORES
    )
    # r: [2, CPAD + NCOL] bf16 — cols [0:128) coef matrix (cf[k, b] =
    # x*(b)[k]), cols [128:) the basis slice for this core's t-range.
    r = nc.declare_dram_parameter("r", [2, CPAD + NCOL], bf, isOutput=False)
    out = nc.declare_dram_parameter(
        "out", [NGRP // 2, P, 2 * GRP], bf, isOutput=True
    )

    with tile.TileContext(nc) as tc:
        with (
            tc.tile_pool(name="consts", bufs=1) as consts,
            tc.psum_pool(name="ps", bufs=4) as psp,
            tc.tile_pool(name="ot", bufs=6) as otp,
        ):
            rt = consts.tile([2, CPAD + NCOL], bf)
            # 3-way load split: group 0/1's matmuls only wait for the tiny
            # first chunk; the rest streams on both HWDGE queues in parallel.
            S1 = CPAD + 2 * GRP
            S2 = S1 + (CPAD + NCOL - S1) // 2
            nc.sync.dma_start(out=rt[:, 0:S1], in_=r[:, 0:S1])
            nc.scalar.dma_start(out=rt[:, S1:S2], in_=r[:, S1:S2])
            nc.sync.dma_start(out=rt[:, S2:], in_=r[:, S2:])
            cf = rt[:, 0:CPAD]

            # Copies stay at GRP granularity (one PSUM group each), but
            # output tiles pair two groups -> 512 KB DMAs (4 KB/partition
            # descriptors, half the sync-queue issue chain).  Within each
            # pair the two copies land on BOTH engines, so the final pair
            # drains concurrently (tail taper).
            o = None
            for g in range(NGRP):
                lo = g * GRP
                ps = psp.tile([P, GRP], f32)
                for c in range(GRP // MM):
                    nc.tensor.matmul(
                        out=ps[:, c * MM : (c + 1) * MM],
                        lhsT=cf,
                        rhs=rt[:, CPAD + lo + c * MM : CPAD + lo + (c + 1) * MM],
                        start=True,
                        stop=True,
                    )
                half = g % 2
                if half == 0:
                    o = otp.tile([P, 2 * GRP], bf)
                dst = o[:, half * GRP : (half + 1) * GRP]
                if pattern[g] == "A":
                    nc.scalar.copy(out=dst, in_=ps[:])
                else:
                    nc.vector.tensor_scalar_mul(dst, ps[:], 1.0)
                if half == 1:
                    nc.sync.dma_start(out=out[g // 2], in_=o[:])
    nc.compile()
    return nc


def _early_phase(dy, x0, cov0, A32):
    """Exact fp32 replica of the reference scan until cov == 0 exactly.

    Returns (early_out (B, t0, 2), xstar (B, 2), t0)."""
    x = x0.astype(np.float32).copy()
    cov = cov0.astype(np.float32).copy()
    rows = []
    t = 0
    while t < T and not np.all(cov == 0):
        rows.append(x * DT32)
        K = A32[None, :, :] - cov
        dx = np.einsum("bij,bj->bi", K, x) * DT32 + np.einsum(
            "bij,bj->bi", cov, dy[:, t, :]
        )
        cov = np.einsum("bij,jk->bik", cov, A32) + np.einsum(
            "ij,bjk->bik", A32, cov
        )
        x = x + dx
        t += 1
    early = (
        np.stack(rows, axis=1) if rows else np.zeros((B, 0, 2), np.float32)
    )
    return early.astype(np.float32), x, t


def _powers(A, n):
    """G^k for k in [0, n), fp64 block products; G = I + dt*A."""
    dtv = float(DT32)
    G = np.eye(2, dtype=np.float64) + dtv * A.astype(np.float64)
    S = 1024
    Ps = np.empty((S, 2, 2), np.float64)
    cur = np.eye(2, dtype=np.float64)
    for s in range(S):
        Ps[s] = cur
        cur = cur @ G
    GS = cur  # G^S
    M = (n + S - 1) // S
    Cs = np.empty((M, 2, 2), np.float64)
    cur = np.eye(2, dtype=np.float64)
    for m in range(M):
        Cs[m] = cur
        cur = cur @ GS
    # G^(m*S + s) = G^(m*S) @ G^s
    return np.einsum("mij,sjk->msik", Cs, Ps).reshape(M * S, 2, 2)[:n]


def kernel(dy, x0, cov0, A):
    global LAST_RESULTS
    from concourse.bass_utils import run_bass_kernel_spmd

    dy = np.ascontiguousarray(np.asarray(dy, dtype=np.float32))
    x0 = np.asarray(x0, dtype=np.float32)
    cov0 = np.asarray(cov0, dtype=np.float32)
    A32 = np.asarray(A, dtype=np.float32)
    assert dy.shape == (B, T, 2) and x0.shape == (B, 2)

    early, xstar, t0 = _early_phase(dy, x0, cov0, A32)
    K = T - t0
    dtv = float(DT32)

    # Basis: RB[k, 2t+i] = dt * (G^(t-t0))[i, k]  for t >= t0, else 0.
    RB = np.zeros((2, 2 * T), np.float64)
    if K > 0:
        Wfull = _powers(A32, K) * dtv          # (K, 2, 2) = dt*G^(t-t0)[i,k]
        RB[0, 2 * t0 :] = Wfull[:, :, 0].reshape(-1)
        RB[1, 2 * t0 :] = Wfull[:, :, 1].reshape(-1)
    RBb = RB.astype(np.float32).astype(BF16)
    cfb = np.ascontiguousarray(
        xstar.T.astype(np.float32).astype(BF16)
    )  # (2, 128)

    if COPY_PATTERN not in _PROGRAMS:
        _PROGRAMS[COPY_PATTERN] = _build_program(COPY_PATTERN)
    nc = _PROGRAMS[COPY_PATTERN]

    in_maps = []
    for c in range(N_CORES):
        plane = np.concatenate(
            [cfb, RBb[:, c * NCOL : (c + 1) * NCOL]], axis=1
        )
        in_maps.append({"r": np.ascontiguousarray(plane)})

    res = run_bass_kernel_spmd(nc, in_maps, list(range(N_CORES)), trace=TRACE)
    LAST_RESULTS = res

    parts = []
    for c in range(N_CORES):
        arr = np.asarray(res.results[c]["out"])  # (NGRP//2, P, 2*GRP) bf16
        parts.append(arr.transpose(1, 0, 2).reshape(P, NCOL))
    full = (
        np.concatenate(parts, axis=1).reshape(B, T, 2).astype(np.float32)
    )
    if t0 > 0:
        full[:, :t0, :] = early
    return np.ascontiguousarray(full)


# revision 27
# speedup vs baseline: 1.0614x; 1.0163x over previous
"""Bass/Trainium2 kernel for the GaussianRecu (Kalman-style linear scan) model.

Reference recursion (C = I, dt = 0.01), per batch b, scanned over t:
    out_t   = dt * x_t                      (emitted before update)
    x_{t+1} = x_t + dt*(A - cov_t) x_t + cov_t dy_t
    cov_{t+1} = cov_t A + A cov_t

The cov recursion is linear with spectral radius 2*rho(A); for contracting A
it underflows to EXACT fp32 zero after a few dozen steps.  Once cov == 0
exactly, the remaining recursion is exactly x <- x + dt*(A x), i.e.
    out[b, t, :] = W_t @ x*(b),   W_t = dt * G^(t-t0),  G = I + dt*A.

Device-side this is a rank-2 broadcast.  The harness tolerance is 2e-2
(max-abs-scaled), so the device pipeline runs in bf16 end to end (measured
error ~5e-3), halving HBM writes (8 MB -> 4 MB per core).

Sharding: TIME-parallel (not batch-parallel).  Each core holds ALL 128
batch rows on the PSUM partition axis and 1/8 of the (t, i) columns:

    psum[b, j] = sum_k coef[k, b] * basis[k, j]

is a K=2 matmul with the tiny [2, 128] coefficient matrix STATIONARY in the
PE array and the basis slice streaming at 1 column/cycle -> 128 output
elements/cycle.  The per-element vector-engine work collapses to a single
PSUM -> SBUF bf16 copy, alternated between ACT and DVE; sync-queue DMAs
stream the copies out.  Per core: ~65 KB input load, 4 MB output write.
"""

import numpy as np
import ml_dtypes

B, T = 128, 65536
DT32 = np.float32(0.01)
N_CORES = 8
P = 128                    # PSUM/SBUF partitions = batch rows
NCOL = 2 * T // N_CORES    # (t, i) columns per core (16384)
MM = 512                   # matmul moving free dim / one PSUM bank (f32)
GRP = 1024                 # columns per PSUM group / output DMA (2 banks)
NGRP = NCOL // GRP         # 16 groups
CPAD = 128                 # coef columns prepended to the basis plane

BF16 = ml_dtypes.bfloat16

TRACE = False          # test harness may set True to collect a HW profile
LAST_RESULTS = None    # BassKernelResults of the most recent device run

# Which engine copies PSUM group g to SBUF: A = ACT (scalar), D = DVE.
COPY_PATTERN = "ADADADADADADADAD"

_PROGRAMS = {}


def _build_program(pattern):
    import concourse.bacc as bacc
    import concourse.tile as tile
    from concourse import mybir

    bf = mybir.dt.bfloat16
    f32 = mybir.dt.float32
    nc = bacc.Bacc(
        "TRN2", target_bir_lowering=False, debug=False, num_devices=N_CORES
    )
    # r: [2, CPAD + NCOL] bf16 — cols [0:128) coef matrix (cf[k, b] =
    # x*(b)[k]), cols [128:) the basis slice for this core's t-range.
    r = nc.declare_dram_parameter("r", [2, CPAD + NCOL], bf, isOutput=False)
    out = nc.declare_dram_parameter("out", [NGRP, P, GRP], bf, isOutput=True)

    with tile.TileContext(nc) as tc:
        with (
            tc.tile_pool(name="consts", bufs=1) as consts,
            tc.psum_pool(name="ps", bufs=4) as psp,
            tc.tile_pool(name="ot", bufs=6) as otp,
        ):
            rt = consts.tile([2, CPAD + NCOL], bf)
            # 3-way load split: group 0/1's matmuls only wait for the tiny
            # first chunk; the rest streams on both HWDGE queues in parallel.
            S1 = CPAD + 2 * GRP
            S2 = S1 + (CPAD + NCOL - S1) // 2
            nc.sync.dma_start(out=rt[:, 0:S1], in_=r[:, 0:S1])
            nc.scalar.dma_start(out=rt[:, S1:S2], in_=r[:, S1:S2])
            nc.sync.dma_start(out=rt[:, S2:], in_=r[:, S2:])
            cf = rt[:, 0:CPAD]

            for g in range(NGRP):
                lo = g * GRP
                ps = psp.tile([P, GRP], f32)
                for c in range(GRP // MM):
                    nc.tensor.matmul(
                        out=ps[:, c * MM : (c + 1) * MM],
                        lhsT=cf,
                        rhs=rt[:, CPAD + lo + c * MM : CPAD + lo + (c + 1) * MM],
                        start=True,
                        stop=True,
                    )
                o = otp.tile([P, GRP], bf)
                if g == NGRP - 1:
                    # Tail taper: copy the two PSUM halves on BOTH engines
                    # concurrently so the post-last-matmul chain shortens.
                    nc.vector.tensor_scalar_mul(o[:, 0:MM], ps[:, 0:MM], 1.0)
                    nc.scalar.copy(out=o[:, MM:GRP], in_=ps[:, MM:GRP])
                elif pattern[g] == "A":
                    nc.scalar.copy(out=o[:], in_=ps[:])
                else:
                    nc.vector.tensor_scalar_mul(o[:], ps[:], 1.0)
                nc.sync.dma_start(out=out[g], in_=o[:])
    nc.compile()
    return nc


def _early_phase(dy, x0, cov0, A32):
    """Exact fp32 replica of the reference scan until cov == 0 exactly.

    Returns (early_out (B, t0, 2), xstar (B, 2), t0)."""
    x = x0.astype(np.float32).copy()
    cov = cov0.astype(np.float32).copy()
    rows = []
    t = 0
    while t < T and not np.all(cov == 0):
        rows.append(x * DT32)
        K = A32[None, :, :] - cov
        dx = np.einsum("bij,bj->bi", K, x) * DT32 + np.einsum(
            "bij,bj->bi", cov, dy[:, t, :]
        )
        cov = np.einsum("bij,jk->bik", cov, A32) + np.einsum(
            "ij,bjk->bik", A32, cov
        )
        x = x + dx
        t += 1
    early = (
        np.stack(rows, axis=1) if rows else np.zeros((B, 0, 2), np.float32)
    )
    return early.astype(np.float32), x, t


def _powers(A, n):
    """G^k for k in [0, n), fp64 block products; G = I + dt*A."""
    dtv = float(DT32)
    G = np.eye(2, dtype=np.float64) + dtv * A.astype(np.float64)
    S = 1024
    Ps = np.empty((S, 2, 2), np.float64)
    cur = np.eye(2, dtype=np.float64)
    for s in range(S):
        Ps[s] = cur
        cur = cur @ G
    GS = cur  # G^S
    M = (n + S - 1) // S
    Cs = np.empty((M, 2, 2), np.float64)
    cur = np.eye(2, dtype=np.float64)
    for m in range(M):
        Cs[m] = cur
        cur = cur @ GS
    # G^(m*S + s) = G^(m*S) @ G^s
    return np.einsum("mij,sjk->msik", Cs, Ps).reshape(M * S, 2, 2)[:n]


def kernel(dy, x0, cov0, A):
    global LAST_RESULTS
    from concourse.bass_utils import run_bass_kernel_spmd

    dy = np.ascontiguousarray(np.asarray(dy, dtype=np.float32))
    x0 = np.asarray(x0, dtype=np.float32)
    cov0 = np.asarray(cov0, dtype=np.float32)
    A32 = np.asarray(A, dtype=np.float32)
    assert dy.shape == (B, T, 2) and x0.shape == (B, 2)

    early, xstar, t0 = _early_phase(dy, x0, cov0, A32)
    K = T - t0
    dtv = float(DT32)

    # Basis: RB[k, 2t+i] = dt * (G^(t-t0))[i, k]  for t >= t0, else 0.
    RB = np.zeros((2, 2 * T), np.float64)
    if K > 0:
        Wfull = _powers(A32, K) * dtv          # (K, 2, 2) = dt*G^(t-t0)[i,k]
        RB[0, 2 * t0 :] = Wfull[:, :, 0].reshape(-1)
        RB[1, 2 * t0 :] = Wfull[:, :, 1].reshape(-1)
    RBb = RB.astype(np.float32).astype(BF16)
    cfb = np.ascontiguousarray(
        xstar.T.astype(np.float32).astype(BF16)
    )  # (2, 128)

    if COPY_PATTERN not in _PROGRAMS:
        _PROGRAMS[COPY_PATTERN] = _build_program(COPY_PATTERN)
    nc = _PROGRAMS[COPY_PATTERN]

    in_maps = []
    for c in range(N_CORES):
        plane = np.concatenate(
            [cfb, RBb[:, c * NCOL : (c + 1) * NCOL]], axis=1
        )
        in_maps.append({"r": np.ascontiguousarray(plane)})

    res = run_bass_kernel_spmd(nc, in_maps, list(range(N_CORES)), trace=TRACE)
    LAST_RESULTS = res

    parts = []
    for c in range(N_CORES):
        arr = np.asarray(res.results[c]["out"])    # (NGRP, P, GRP) bf16
        parts.append(arr.transpose(1, 0, 2).reshape(P, NCOL))
    full = (
        np.concatenate(parts, axis=1).reshape(B, T, 2).astype(np.float32)
    )
    if t0 > 0:
        full[:, :t0, :] = early
    return np.ascontiguousarray(full)


# revision 29
# speedup vs baseline: 1.0629x; 1.0014x over previous
"""Bass/Trainium2 kernel for the GaussianRecu (Kalman-style linear scan) model.

Reference recursion (C = I, dt = 0.01), per batch b, scanned over t:
    out_t   = dt * x_t                      (emitted before update)
    x_{t+1} = x_t + dt*(A - cov_t) x_t + cov_t dy_t
    cov_{t+1} = cov_t A + A cov_t

The cov recursion is linear with spectral radius 2*rho(A); for contracting A
it underflows to EXACT fp32 zero after a few dozen steps.  Once cov == 0
exactly, the remaining recursion is exactly x <- x + dt*(A x), i.e.
    out[b, t, :] = W_t @ x*(b),   W_t = dt * G^(t-t0),  G = I + dt*A.

Device-side this is a rank-2 broadcast.  The harness tolerance is 2e-2
(max-abs-scaled), so the device pipeline runs in bf16 end to end (measured
error ~5e-3), halving HBM writes (8 MB -> 4 MB per core).

Sharding: TIME-parallel (not batch-parallel).  Each core holds ALL 128
batch rows on the PSUM partition axis and 1/8 of the (t, i) columns:

    psum[b, j] = sum_k coef[k, b] * basis[k, j]

is a K=2 matmul with the tiny [2, 128] coefficient matrix STATIONARY in the
PE array and the basis slice streaming at 1 column/cycle -> 128 output
elements/cycle.  The per-element vector-engine work collapses to a single
PSUM -> SBUF bf16 copy, alternated between ACT and DVE; sync-queue DMAs
stream the copies out.  Per core: ~65 KB input load, 4 MB output write.
"""

import numpy as np
import ml_dtypes

B, T = 128, 65536
DT32 = np.float32(0.01)
N_CORES = 8
P = 128                    # PSUM/SBUF partitions = batch rows
NCOL = 2 * T // N_CORES    # (t, i) columns per core (16384)
MM = 512                   # matmul moving free dim / one PSUM bank (f32)
GRP = 1024                 # columns per PSUM group / output DMA (2 banks)
NGRP = NCOL // GRP         # 16 groups
CPAD = 128                 # coef columns prepended to the basis plane

BF16 = ml_dtypes.bfloat16

TRACE = False          # test harness may set True to collect a HW profile
LAST_RESULTS = None    # BassKernelResults of the most recent device run

# Which engine copies PSUM group g to SBUF: A = ACT (scalar), D = DVE.
COPY_PATTERN = "DADADADADADADADA"

_PROGRAMS = {}


def _build_program(pattern):
    import concourse.bacc as bacc
    import concourse.tile as tile
    from concourse import mybir

    bf = mybir.dt.bfloat16
    f32 = mybir.dt.float32
    nc = bacc.Bacc(
        "TRN2", target_bir_lowering=False, debug=False, num_devices=N_CORES
    )
    # r: [2, CPAD + NCOL] bf16 — cols [0:128) coef matrix (cf[k, b] =
    # x*(b)[k]), cols [128:) the basis slice for this core's t-range.
    r = nc.declare_dram_parameter("r", [2, CPAD + NCOL], bf, isOutput=False)
    out = nc.declare_dram_parameter("out", [NGRP, P, GRP], bf, isOutput=True)

    with tile.TileContext(nc) as tc:
        with (
            tc.tile_pool(name="consts", bufs=1) as consts,
            tc.psum_pool(name="ps", bufs=4) as psp,
            tc.tile_pool(name="ot", bufs=6) as otp,
        ):
            rt = consts.tile([2, CPAD + NCOL], bf)
            # 3-way load split: group 0/1's matmuls only wait for the tiny
            # first chunk; the rest streams on both HWDGE queues in parallel.
            S1 = CPAD + 2 * GRP
            S2 = S1 + (CPAD + NCOL - S1) // 2
            nc.sync.dma_start(out=rt[:, 0:S1], in_=r[:, 0:S1])
            nc.scalar.dma_start(out=rt[:, S1:S2], in_=r[:, S1:S2])
            nc.sync.dma_start(out=rt[:, S2:], in_=r[:, S2:])
            cf = rt[:, 0:CPAD]

            for g in range(NGRP):
                lo = g * GRP
                ps = psp.tile([P, GRP], f32)
                for c in range(GRP // MM):
                    nc.tensor.matmul(
                        out=ps[:, c * MM : (c + 1) * MM],
                        lhsT=cf,
                        rhs=rt[:, CPAD + lo + c * MM : CPAD + lo + (c + 1) * MM],
                        start=True,
                        stop=True,
                    )
                o = otp.tile([P, GRP], bf)
                if g == NGRP - 1:
                    # Tail taper: copy the two PSUM halves on BOTH engines
                    # concurrently, and drain each half on its own HWDGE
                    # ring, so the post-last-matmul chain shortens.
                    nc.vector.tensor_scalar_mul(o[:, 0:MM], ps[:, 0:MM], 1.0)
                    nc.scalar.copy(out=o[:, MM:GRP], in_=ps[:, MM:GRP])
                    nc.sync.dma_start(out=out[g][:, 0:MM], in_=o[:, 0:MM])
                    nc.scalar.dma_start(
                        out=out[g][:, MM:GRP], in_=o[:, MM:GRP]
                    )
                else:
                    if pattern[g] == "A":
                        nc.scalar.copy(out=o[:], in_=ps[:])
                    else:
                        nc.vector.tensor_scalar_mul(o[:], ps[:], 1.0)
                    nc.sync.dma_start(out=out[g], in_=o[:])
    nc.compile()
    return nc


def _early_phase(dy, x0, cov0, A32):
    """Exact fp32 replica of the reference scan until cov == 0 exactly.

    Returns (early_out (B, t0, 2), xstar (B, 2), t0)."""
    x = x0.astype(np.float32).copy()
    cov = cov0.astype(np.float32).copy()
    rows = []
    t = 0
    while t < T and not np.all(cov == 0):
        rows.append(x * DT32)
        K = A32[None, :, :] - cov
        dx = np.einsum("bij,bj->bi", K, x) * DT32 + np.einsum(
            "bij,bj->bi", cov, dy[:, t, :]
        )
        cov = np.einsum("bij,jk->bik", cov, A32) + np.einsum(
            "ij,bjk->bik", A32, cov
        )
        x = x + dx
        t += 1
    early = (
        np.stack(rows, axis=1) if rows else np.zeros((B, 0, 2), np.float32)
    )
    return early.astype(np.float32), x, t


def _powers(A, n):
    """G^k for k in [0, n), fp64 block products; G = I + dt*A."""
    dtv = float(DT32)
    G = np.eye(2, dtype=np.float64) + dtv * A.astype(np.float64)
    S = 1024
    Ps = np.empty((S, 2, 2), np.float64)
    cur = np.eye(2, dtype=np.float64)
    for s in range(S):
        Ps[s] = cur
        cur = cur @ G
    GS = cur  # G^S
    M = (n + S - 1) // S
    Cs = np.empty((M, 2, 2), np.float64)
    cur = np.eye(2, dtype=np.float64)
    for m in range(M):
        Cs[m] = cur
        cur = cur @ GS
    # G^(m*S + s) = G^(m*S) @ G^s
    return np.einsum("mij,sjk->msik", Cs, Ps).reshape(M * S, 2, 2)[:n]


def kernel(dy, x0, cov0, A):
    global LAST_RESULTS
    from concourse.bass_utils import run_bass_kernel_spmd

    dy = np.ascontiguousarray(np.asarray(dy, dtype=np.float32))
    x0 = np.asarray(x0, dtype=np.float32)
    cov0 = np.asarray(cov0, dtype=np.float32)
    A32 = np.asarray(A, dtype=np.float32)
    assert dy.shape == (B, T, 2) and x0.shape == (B, 2)

    early, xstar, t0 = _early_phase(dy, x0, cov0, A32)
    K = T - t0
    dtv = float(DT32)

    # Basis: RB[k, 2t+i] = dt * (G^(t-t0))[i, k]  for t >= t0, else 0.
    RB = np.zeros((2, 2 * T), np.float64)
    if K > 0:
        Wfull = _powers(A32, K) * dtv          # (K, 2, 2) = dt*G^(t-t0)[i,k]
        RB[0, 2 * t0 :] = Wfull[:, :, 0].reshape(-1)
        RB[1, 2 * t0 :] = Wfull[:, :, 1].reshape(-1)
    RBb = RB.astype(np.float32).astype(BF16)
    cfb = np.ascontiguousarray(
        xstar.T.astype(np.float32).astype(BF16)
    )  # (2, 128)

    if COPY_PATTERN not in _PROGRAMS:
        _PROGRAMS[COPY_PATTERN] = _build_program(COPY_PATTERN)
    nc = _PROGRAMS[COPY_PATTERN]

    in_maps = []
    for c in range(N_CORES):
        plane = np.concatenate(
            [cfb, RBb[:, c * NCOL : (c + 1) * NCOL]], axis=1
        )
        in_maps.append({"r": np.ascontiguousarray(plane)})

    res = run_bass_kernel_spmd(nc, in_maps, list(range(N_CORES)), trace=TRACE)
    LAST_RESULTS = res

    parts = []
    for c in range(N_CORES):
        arr = np.asarray(res.results[c]["out"])    # (NGRP, P, GRP) bf16
        parts.append(arr.transpose(1, 0, 2).reshape(P, NCOL))
    full = (
        np.concatenate(parts, axis=1).reshape(B, T, 2).astype(np.float32)
    )
    if t0 > 0:
        full[:, :t0, :] = early
    return np.ascontiguousarray(full)
